# revision 15
# baseline (speedup 1.0000x reference)
"""Trainium2 Bass kernel for: out = (x @ wsums.sum(0)) * (1.5 * 0.5).

x: [1024, 8192] f32, wsums: [32, 8192] f32 -> out: [1024, 1] f32.

Sharding across 8 NeuronCores: 8-way along the contraction dim k
(8192 -> 1024 per core).  The host pre-reduces wsums to
wt = SCALE * wsums.sum(0), casts both x and wt to bf16, and ships each
core its [1024, 1024] x column-shard (2MB) plus a [1, 1024] wt k-slice.
The host sums the 8 per-core partials (the unshard step for a
contraction-sharded dim).  bf16 inputs halve the HBM stream (the
binding resource for this memory-bound problem) and put the DVE in its
2x packed mode; the error budget (~0.3% from input rounding) sits far
under the 2e-2 correctness gate.

Layout: x rows are PAIRED per partition — "dblock" d covers rows
[256d, 256d+256) with partition p holding rows 256d+2p and 256d+2p+1 as
one contiguous 4KB HBM read (bf16 2KB rows would otherwise make the
DMA packet-bound: 2KB packets run ~110ns vs 4KB at ~158ns per SDMA
engine).  acc[p, 2d+r] = dot of row 256d+2p+r.

Every DMA covers all 128 partitions: partition-sliced DMAs (e.g.
out=xt[0:92]) make walrus assign descriptors in contiguous-partition
chunks to a handful of SDMA engines (measured: a [0:92] piece put 46
packets each on 4 engines and zero on the rest), destroying the
16-engine spread.  Free-dim slicing keeps the spread uniform, so the
first and last dblocks are split into their two row-columns ([:, 0:KB]
and [:, KB:2KB]) — early compute start and a short tail — while the
middle dblocks stay whole for 4KB descriptors.

Per-core device program:
  1. wp[128, KB] bf16 <- one partition-stride-0 broadcast DMA of wt,
     FIRST on the SYNC queue (a scalar-queue wp was served ~2.3us late
     and gated the first multiply).  No PE matmul, no PSUM: the Tensor
     engine never runs.
  2. x streamed as 6 DMAs on the sync queue (d0 halves, d1, d2, d3
     halves), all tiles SBUF-resident so every DMA is triggered
     up-front and the stream runs at line rate.
  3. Multiplies all on DVE (bf16 2x packed mode; wp rides a stride-0
     middle dim for whole-dblock multiplies).  Accumulates: ScalarE
     activation+accum_out for rows 0-5 (1 elem/cycle/lane, runs in
     parallel with the DVE), DVE for rows 6-7 via an add-halves tree
     (bf16 2x) plus one paired 1x tensor_reduce into bf16 acc (fp32
     internal accumulation in both engines; only the final write
     rounds).  GpSimd is kept OFF the compute path: its SBUF port is
     shared with the DVE and any GpSimd activity halves DVE multiply
     throughput (measured 1789ns vs 679ns per [128,1024] bf16 TT).
  4. DMA the [128, 8] bf16 accumulator block to DRAM.

Environment workarounds (this container's walrus build):
  - it encodes at most ONE semaphore wait per instruction ("Too many sync
    wait commands"), so compile_bir_kernel is wrapped with a BIR post-pass
    that moves excess waits onto preceding same-engine NoOp instructions;
  - it cannot encode bass_isa raw-ISA ops (tensor_tensor_reduce,
    affine_mul_reduce, partition_all_reduce, ... -> "ISA wrong length"),
    so only classic mybir ops are used (TensorTensor / Activation /
    TensorReduce / DMACopy).
"""

import json

import ml_dtypes
import numpy as np

import concourse.bass as bass
import concourse.bass2jax as bass2jax
import concourse.bass_utils as bass_utils
import concourse.mybir as mybir
from concourse.tile import TileContext

SCALE = 1.5 * 0.5
B, K, G = 1024, 8192, 32
N_CORES = 8
KSHARD = 8                  # cores along k
KB = K // KSHARD            # per-core k width
P = 128
NDBLK = B // (2 * P)        # double-row blocks per core (4)
BF16 = mybir.dt.bfloat16

# Set by test.py to profile; results stashed in LAST_RESULTS.
TRACE = False
TRACE_KWARGS = {}
LAST_RESULTS = None

_built = None

# ---------------------------------------------------------------------------
# Workaround: this container's walrus encodes at most 1 sync wait per
# instruction.  Split longer on_wait lists onto preceding same-engine NoOps.
MAX_WAITS = 1
_orig_compile_bir_kernel = bass_utils.compile_bir_kernel


def _split_waits_in_bir(bir: dict) -> int:
    counter = [0]

    def fix_blocks(blocks):
        for bb in blocks:
            out = []
            for ins in bb.get("instructions", []):
                si = ins.get("sync_info")
                ow = (si or {}).get("on_wait") or []
                if len(ow) > MAX_WAITS:
                    extra, keep = ow[:-MAX_WAITS], ow[-MAX_WAITS:]
                    for i in range(0, len(extra), MAX_WAITS):
                        counter[0] += 1
                        out.append({
                            "name": f"I-waitsplit-{counter[0]}",
                            "engine": ins["engine"],
                            "opcode": "NoOp",
                            "ins": [],
                            "outs": [],
                            "debug": ins.get("debug", 0),
                            "sync_info": {
                                "on_update": [],
                                "on_wait": extra[i : i + MAX_WAITS],
                            },
                        })
                    si["on_wait"] = keep
                out.append(ins)
            bb["instructions"] = out
            if bb.get("blocks"):
                fix_blocks(bb["blocks"])

    for fn in bir["functions"]:
        fix_blocks(fn["blocks"])
    return counter[0]


def _patched_compile_bir_kernel(bir_json, tmpdir, neff_name="file.neff"):
    if isinstance(bir_json, str):
        bir_json = bir_json.encode()
    bir = json.loads(bir_json)
    _split_waits_in_bir(bir)
    return _orig_compile_bir_kernel(json.dumps(bir).encode(), tmpdir, neff_name)


bass_utils.compile_bir_kernel = _patched_compile_bir_kernel
bass2jax.compile_bir_kernel = _patched_compile_bir_kernel


# ---------------------------------------------------------------------------
# Overlapped TileContext exit.  The stock exit serializes: drain(+DMA-sem
# waits) -> all-engine barrier -> sem clears -> barrier, so every engine's
# ~3-6us walrus postamble (each engine zeroes a fixed 51-semaphore slice:
# Tensor S[3-53], Scalar S[54-104], GpSimd S[105-155], Vector S[156-206],
# Sync S[207-255]) starts only after the out-DMA's ~2us completion receipt.
# This kernel's live semaphores (Tile range ~151-174: barrier, engine
# clocks, DMAHW lanes) fall ONLY in the GpSimd and Vector slices, so:
#   - Tensor and Scalar get no tail instructions at all -> their postambles
#     run as soon as their body ends;
#   - Sync drains with the global-clock + DMA-completion waits, then incs a
#     handoff semaphore;
#   - GpSimd and Vector wait for the handoff before entering their
#     postambles (so the DMAHW/clock sems they zero are no longer in use).
# The explicit Tile sem clears are dropped: the walrus postamble wipes all
# 256 semaphores every execution, which keeps re-execution correct.
import concourse.tile as tile_mod
from concourse.tile import TileContext as _TC


def _overlap_drain_and_barrier(self, tick_clock, wait_clock):
    nc = self.nc
    drain_inst = nc.sync.drain()
    wait_clock.add_sem_waits(
        drain_inst.ins,
        tile_mod.ScopedClock({None: tick_clock.global_clock}),
    )
    done = nc.alloc_semaphore("tail_dma_done")
    # Must not sit in Tensor's or Scalar's postamble-clear slice (they are
    # released early and would zero it while GpSimd/Vector still wait).
    assert done.num >= 105, done.num
    drain_inst.then_inc(done, 1)
    nc.gpsimd.wait_ge(done, 1)
    nc.vector.wait_ge(done, 1)
    popped = nc._tile_sem_poison_stack.pop()
    assert popped is self._sem_poison


_TC._drain_and_barrier = _overlap_drain_and_barrier
# ---------------------------------------------------------------------------


def _build():
    # Bass.__init__ ends with an all-engine barrier ordering its const-AP
    # memsets (fp32 0/1, bf16 1, u8 127) against the body.  This kernel
    # never reads those const APs, and the NRT start barrier already aligns
    # the engines at execution start, so skip it: Sync reaches the first
    # DMA trigger ~1.7us earlier.
    _orig_aeb = bass.Bass.all_engine_barrier
    bass.Bass.all_engine_barrier = lambda self, **kw: None
    try:
        nc = bass.Bass("TRN2")
    finally:
        bass.Bass.all_engine_barrier = _orig_aeb
    x_sh = nc.dram_tensor("x_shard", (B, KB), BF16, kind="ExternalInput")
    wt = nc.dram_tensor("wt_shard", (1, KB), BF16, kind="ExternalInput")
    # Padded to 512B/partition: sub-512B DRAM writes do read-modify-write
    # in the SDMA engines (a [P, 8] bf16 out took ~2.7us); host reads
    # cols 0:8.
    out = nc.dram_tensor("out_acc", (P, 256), BF16, kind="ExternalOutput")

    H = KB // 2
    with TileContext(nc) as tc, nc.allow_low_precision(
        reason="bf16 partials; fp32 internal accum, 2e-2 gate"
    ):
        with (
            tc.tile_pool(name="const", bufs=1) as cpool,
            tc.tile_pool(name="xbuf", bufs=1) as xpool,
            tc.tile_pool(name="ybuf", bufs=1) as ypool,
        ):
            # wp via partition-stride-0 broadcast DMA, FIRST on the sync
            # queue: every partition reads the same 2KB wt row from DRAM.
            # (A scalar-queue wp was tried: its packets get serviced
            # ~1.5us after the doorbell and straggle through the x
            # stream — net loss.)
            wp = cpool.tile([P, KB], BF16)
            nc.sync.dma_start(
                out=wp, in_=bass.AP(wt, 0, [[0, P], [1, KB]])
            )

            acc = cpool.tile([P, 256], BF16)

            # Stream [1, 2, 1, 2, 2] rows per partition; row map:
            #   xa: rows 0-127    (p -> row p)          -> acc col 0
            #   xb: rows 128-383  (p -> 128+2p+r)       -> acc cols 1,2
            #   xd: rows 384-511  (p -> 384+p)          -> acc col 7
            #   xq1: rows 512-767 (p -> 512+2p+r)       -> acc cols 3,4
            #   xq2: rows 768-1023 (p -> 768+2p+r)      -> acc cols 5,6
            # xd sits mid-stream so its accumulate (a ScalarE slot) isn't
            # jammed into the tail.
            xa = xpool.tile([P, KB], BF16, name="xa")
            nc.sync.dma_start(out=xa, in_=bass.AP(x_sh, 0, [[KB, P], [1, KB]]))
            xb = xpool.tile([P, 2 * KB], BF16, name="xb")
            nc.sync.dma_start(
                out=xb,
                in_=bass.AP(x_sh, 128 * KB, [[2 * KB, P], [1, 2 * KB]]),
            )
            xd = xpool.tile([P, KB], BF16, name="xd")
            nc.sync.dma_start(
                out=xd, in_=bass.AP(x_sh, 384 * KB, [[KB, P], [1, KB]])
            )
            xq1 = xpool.tile([P, 2 * KB], BF16, name="xq1")
            nc.sync.dma_start(
                out=xq1,
                in_=bass.AP(x_sh, 512 * KB, [[2 * KB, P], [1, 2 * KB]]),
            )
            xq2 = xpool.tile([P, 2 * KB], BF16, name="xq2")
            nc.sync.dma_start(
                out=xq2,
                in_=bass.AP(x_sh, 768 * KB, [[2 * KB, P], [1, 2 * KB]]),
            )

            ya = ypool.tile([P, KB], BF16, name="ya")
            yb = ypool.tile([P, 2 * KB], BF16, name="yb")
            yd = ypool.tile([P, KB], BF16, name="yd")
            yq1 = ypool.tile([P, 2 * KB], BF16, name="yq1")
            yq2 = ypool.tile([P, 2 * KB], BF16, name="yq2")
            z7 = cpool.tile([P, H], BF16)
            z56 = cpool.tile([P, KB], BF16)   # viewed [P, 2, 512]

            def tt1(yt, xt):
                nc.vector.tensor_tensor(yt, xt, wp, op=mybir.AluOpType.mult)

            def tt2(yt, xt):
                x3 = xt[:].rearrange("p (a k) -> p a k", a=2)
                y3 = yt[:].rearrange("p (a k) -> p a k", a=2)
                wb = wp[:].unsqueeze(1).broadcast_to([P, 2, KB])
                nc.vector.tensor_tensor(y3, x3, wb, op=mybir.AluOpType.mult)

            def act_col(yt, lo, col, width=KB):
                nc.scalar.activation(
                    yt[:, lo : lo + width],
                    yt[:, lo : lo + width],
                    mybir.ActivationFunctionType.Copy,
                    accum_out=acc[:, col : col + 1],
                )

            # Schedule.  DVE: all multiplies + the row-7 half-add + the
            # rows-5/6 pair tree.  Scalar: rows 0-4 full-width and row 7
            # on its DVE-halved z7.
            tt1(ya, xa)                 # row 0 mult           (DVE)
            act_col(ya, 0, 0)           # col 0                (Scalar)
            tt2(yb, xb)                 # rows 1+2 mult        (DVE)
            act_col(yb, 0, 1)           # col 1                (Scalar)
            act_col(yb, KB, 2)          # col 2                (Scalar)
            tt1(yd, xd)                 # row 7 mult           (DVE)
            nc.vector.tensor_tensor(    # halve row 7          (DVE)
                z7, yd[:, 0:H], yd[:, H:KB], op=mybir.AluOpType.add
            )
            nc.vector.tensor_tensor(    # quarter row 7        (DVE)
                z7[:, 0 : H // 2], z7[:, 0 : H // 2], z7[:, H // 2 : H],
                op=mybir.AluOpType.add,
            )
            act_col(z7, 0, 7, width=H // 2)  # col 7 on z7     (Scalar)
            tt2(yq1, xq1)               # rows 3+4 mult        (DVE)
            act_col(yq1, 0, 3)          # col 3                (Scalar)
            act_col(yq1, KB, 4)         # col 4                (Scalar)
            tt2(yq2, xq2)               # rows 5+6 mult        (DVE)
            y3 = yq2[:].rearrange("p (a k) -> p a k", a=2)
            z3 = z56[:].rearrange("p (a k) -> p a k", a=2)
            nc.vector.tensor_tensor(    # halve rows 5+6       (DVE)
                z3, y3[:, :, 0:H], y3[:, :, H:KB], op=mybir.AluOpType.add
            )
            Q = H // 2
            nc.vector.tensor_tensor(    # quarter rows 5+6     (DVE)
                z3[:, :, 0:Q], z3[:, :, 0:Q], z3[:, :, Q:H],
                op=mybir.AluOpType.add,
            )
            nc.vector.tensor_reduce(    # cols 5,6             (DVE)
                acc[:, 5:7], z3[:, :, 0:Q], axis=mybir.AxisListType.X,
                op=mybir.AluOpType.add,
            )

            nc.sync.dma_start(out=out.ap(), in_=acc)
    return nc


def kernel(x: np.ndarray, wsums: np.ndarray) -> np.ndarray:
    global _built, LAST_RESULTS
    if _built is None:
        _built = _build()
    nc = _built

    x = np.asarray(x, dtype=np.float32)
    wsums = np.asarray(wsums, dtype=np.float32)

    w_total = wsums.sum(axis=0, dtype=np.float32)          # [K]
    wt_full = (w_total * SCALE).astype(np.float32)         # SCALE folded in
    x16 = x.astype(ml_dtypes.bfloat16)
    wt16 = wt_full.astype(ml_dtypes.bfloat16)

    in_maps = []
    for c in range(N_CORES):
        xs = np.ascontiguousarray(x16[:, c * KB : (c + 1) * KB])
        wsl = np.ascontiguousarray(wt16[c * KB : (c + 1) * KB])[None, :]
        in_maps.append({"x_shard": xs, "wt_shard": wsl})

    res = bass_utils.run_bass_kernel_spmd(
        nc,
        in_maps,
        core_ids=list(range(N_CORES)),
        trace=TRACE,
        **TRACE_KWARGS,
    )
    LAST_RESULTS = res

    tot = None
    for c in range(N_CORES):
        acc = res.results[c]["out_acc"][:, 0:8].astype(np.float32)  # [P, 8]
        vec = np.empty(B, dtype=np.float32)
        vec[0:128] = acc[:, 0]
        vec[128:384] = acc[:, 1:3].reshape(2 * P)
        vec[384:512] = acc[:, 7]
        vec[512:768] = acc[:, 3:5].reshape(2 * P)
        vec[768:1024] = acc[:, 5:7].reshape(2 * P)
        tot = vec if tot is None else tot + vec
    return tot.astype(np.float32)[:, None]
